# revision 45
# baseline (speedup 1.0000x reference)
"""GQA attention kernel for 8 Trainium2 NeuronCores (tensor-parallel over heads).

Self-contained: hardcodes shapes from the problem spec.
  x  [2, 1024, 4096]  Wq [4096, 4096]  Wk/Wv [4096, 1024]  Wo [4096, 4096]  bo [4096]
  32 q heads, 8 KV groups, head dim 128, RoPE theta 10000, causal softmax.

Sharding: core c owns KV group c and query heads 4c..4c+3.
  - x is replicated to every core host-side (no on-device AllGather)
  - Wq/Wk/Wv column-sharded per head group (RoPE-permuted, scale folded into Wq)
  - attention computed locally per core; stage 1 and stage 2 interleave per
    batch so the b=0 attnT AllGather overlaps b=1 compute
  - attnT shards AllGathered on-device (one [512,1024] gather per batch),
    Wo column-sharded -> each core returns its 512 output columns
    (transposed); host concatenates, transposes, adds bo.
"""

import numpy as np

import concourse.mybir as mybir
import concourse.tile as tile
from concourse import bacc
from concourse import bass_utils

N_CORES = 8
B, T, C = 2, 1024, 4096
H, G, D = 32, 8, 128
REP = H // G            # q heads per KV group/core = 4
BT = B * T              # 2048
HD_SHARD = REP * D      # 512 q columns per core
CO_SHARD = C // N_CORES  # 512 output columns per core
ROPE_THETA = 10000.0

F32 = mybir.dt.float32
F16 = mybir.dt.float16

_CACHE = {}


def _balanced_chunks(total, maxc=512, quantum=128):
    """Split `total` (multiple of quantum) into chunks <= maxc, balanced."""
    nblk = total // quantum
    n = -(-total // maxc)
    sizes = []
    for i in range(n):
        take = nblk // n + (1 if i < nblk % n else 0)
        sizes.append(take * quantum)
    return sizes


def _build_nc(loop_n=1):
    """Build the kernel. loop_n > 1 wraps the whole body in a hardware
    For_i loop (full all-engine barrier on the back edge) so one dispatch
    runs the kernel loop_n times back-to-back -- used for on-device
    timing; the computed output is identical (idempotent body)."""
    nc = bacc.Bacc("TRN2", target_bir_lowering=False, debug=False, num_devices=N_CORES)

    # ---- I/O ----
    xT = nc.dram_tensor("xT", [C, BT], F16, kind="ExternalInput")
    wq = nc.dram_tensor("wq", [C, HD_SHARD], F16, kind="ExternalInput")
    wk = nc.dram_tensor("wk", [C, D], F16, kind="ExternalInput")
    wv = nc.dram_tensor("wv", [C, D], F16, kind="ExternalInput")
    wo = nc.dram_tensor("wo", [H * D, CO_SHARD], F16, kind="ExternalInput")
    cos2 = nc.dram_tensor("cos2", [D, T], F16, kind="ExternalInput")
    sinpm = nc.dram_tensor("sinpm", [D, T], F16, kind="ExternalInput")
    tri = nc.dram_tensor("tri", [128, 128], F16, kind="ExternalInput")
    idin = nc.dram_tensor("idin", [128, 128], F16, kind="ExternalInput")
    # output stored transposed [co, t]; host transposes back (cheap)
    outT = nc.dram_tensor("outT", [CO_SHARD, BT], F16, kind="ExternalOutput")

    # ---- DRAM scratch ----
    TG = 1024
    NTG = BT // TG  # 2 (one AllGather per batch)
    attnT_dram = [
        nc.dram_tensor(f"attnT_dram_{g}", [HD_SHARD, TG], F16, kind="Internal")
        for g in range(NTG)
    ]
    attnT_full = [
        nc.dram_tensor(f"attnT_full_{g}", [H * D, TG], F16, kind="Internal")
        for g in range(NTG)
    ]

    with tile.TileContext(nc) as tc:
      for it in range(loop_n):  # python-unrolled repeat for on-device timing

        # ================= Stage 1: QKV projection =================
        # qkvT[m, t] = sum_c W[c, m] * xT[c, t];  m = q(512) | k(128) | v(128)
        M_ALL = HD_SHARD + 2 * D  # 768
        M_CHUNKS = M_ALL // 128   # 6 (0-3 q heads, 4 k, 5 v)
        KSUB = C // 128           # 32
        NT = 512                  # t-cols per chunk
        NCH = T // 128  # 8 chunks of both s and t
        with tc.tile_pool(name=f"s1_w{it}", bufs=1) as s1_w:
            # QKV weights stay resident across both batch halves.
            # Per-ko contiguous DMAs spread the load across DMA engines.
            w_sb = s1_w.tile([128, KSUB, M_ALL], F16)
            for k in range(KSUB):
                nc.sync.dma_start(w_sb[:, k, 0:HD_SHARD],
                                  wq[k * 128:(k + 1) * 128, :])
                nc.sync.dma_start(w_sb[:, k, HD_SHARD:HD_SHARD + D],
                                  wk[k * 128:(k + 1) * 128, :])
                nc.sync.dma_start(w_sb[:, k, HD_SHARD + D:M_ALL],
                                  wv[k * 128:(k + 1) * 128, :])
            # qkv stay SBUF-resident between stage 1 and stage 2 (per batch):
            # q heads + k in qk_sb rows [m], v in vT_b. No DRAM round trip.
            qk_sbs = [s1_w.tile([128, REP + 1, T], F16, name=f"qk_{it}_{b}")
                      for b in range(B)]
            vT_bs = [s1_w.tile([128, T], F16, name=f"vT_{it}_{b}")
                     for b in range(B)]

            # Interleave stage 1 and stage 2 per batch: s1(b0) s2(b0) s1(b1)
            # s2(b1).  The b=0 attnT AllGathers then overlap s1(b1)+s2(b1)
            # compute, and stage-2 DVE/ACT work spreads under stage-1 PE.
            for b in range(B):
                # ---- Stage 1 half: QKV projection for this batch ----
                qk_sb = qk_sbs[b]
                vT_b = vT_bs[b]
                with (
                    tc.tile_pool(name=f"s1_x{it}_{b}", bufs=3) as s1_x,
                    tc.tile_pool(name=f"s1_ps{it}_{b}", bufs=8, space="PSUM") as s1_psum,
                ):
                    for n in (2 * b, 2 * b + 1):  # 512-col t chunks
                        psums = [
                            s1_psum.tile([128, NT], F32, name=f"s1ps_{it}_{b}_{m}", tag="s1ps")
                            for m in range(M_CHUNKS)
                        ]
                        for khalf in range(2):
                            x_sb = s1_x.tile([128, KSUB // 2, NT], F16, tag="s1x")
                            for k in range(KSUB // 2):
                                kk = khalf * 16 + k
                                nc.sync.dma_start(
                                    x_sb[:, k, :],
                                    xT[kk * 128:(kk + 1) * 128, n * NT:(n + 1) * NT],
                                )
                            for m in range(M_CHUNKS):
                                for k in range(KSUB // 2):
                                    nc.tensor.matmul(
                                        psums[m][:],
                                        w_sb[:, khalf * 16 + k, m * 128:(m + 1) * 128],
                                        x_sb[:, k, :],
                                        start=(khalf == 0 and k == 0),
                                        stop=(khalf == 1 and k == KSUB // 2 - 1),
                                    )
                        nh = (n % 2) * NT  # col offset within this batch's window
                        for m in range(M_CHUNKS):
                            if m < M_CHUNKS - 1:
                                nc.any.tensor_copy(
                                    qk_sb[:, m, nh:nh + NT], psums[m][:])
                            else:
                                nc.any.tensor_copy(
                                    vT_b[:, nh:nh + NT], psums[m][:])

                # ---- Stage 2: attention for this batch ----
                with (
                    tc.tile_pool(name=f"s2_const{it}_{b}", bufs=1) as s2_const,
                    tc.tile_pool(name=f"s2_kv{it}_{b}", bufs=2) as s2_kv,
                    tc.tile_pool(name=f"s2_q{it}_{b}", bufs=3) as s2_q,
                    tc.tile_pool(name=f"s2_tmp{it}_{b}", bufs=3) as s2_tmp,
                    tc.tile_pool(name=f"s2_probs{it}_{b}", bufs=3) as s2_probs,
                    tc.tile_pool(name=f"s2_out{it}_{b}", bufs=3) as s2_out,
                    tc.tile_pool(name=f"s2_ps_sc{it}_{b}", bufs=3, space="PSUM") as s2_ps_sc,
                    tc.tile_pool(name=f"s2_ps_pv{it}_{b}", bufs=2, space="PSUM") as s2_ps_pv,
                    tc.tile_pool(name=f"s2_ps_tr{it}_{b}", bufs=2, space="PSUM") as s2_ps_tr,
                ):
                    ident = s2_const.tile([128, 128], F16)
                    nc.sync.dma_start(ident[:], idin[:])
                    tri_sb = s2_const.tile([128, 128], F16)
                    nc.sync.dma_start(tri_sb[:], tri[:])
                    cos_sb = s2_const.tile([128, T], F16)
                    nc.sync.dma_start(cos_sb[:], cos2[:])
                    sin_sb = s2_const.tile([128, T], F16)
                    nc.sync.dma_start(sin_sb[:], sinpm[:])

                    def load_rope(dst, head):
                        """Rope head rows of SBUF-resident qk_sb into dst.

                        dst = src * [cos;cos] + swap_halves(src) * [-sin;+sin]
                        swap via SBUF->SBUF DMA (crosses partitions).
                        """
                        swp = s2_tmp.tile([128, T], F16, tag="rope_swp")
                        nc.sync.dma_start(swp[0:64], qk_sb[64:128, head, :])
                        nc.sync.dma_start(swp[64:128], qk_sb[0:64, head, :])
                        tmp = s2_tmp.tile([128, T], F16, tag="rope_tmp")
                        nc.vector.tensor_tensor(tmp[:], swp[:], sin_sb[:], mybir.AluOpType.mult)
                        nc.vector.tensor_tensor(dst[:], qk_sb[:, head, :], cos_sb[:], mybir.AluOpType.mult)
                        nc.vector.tensor_tensor(dst[:], dst[:], tmp[:], mybir.AluOpType.add)

                    # k rope
                    k_rope = s2_kv.tile([128, T], F16, tag="k_rope")
                    load_rope(k_rope, REP)
                    # v: transpose SBUF-resident vT to [s, d], append ones col
                    v_sb = s2_kv.tile([128, NCH, D + 1], F16, tag="v_ext")
                    nc.vector.memset(v_sb[:, :, D:D + 1], 1.0)
                    for j in range(NCH):
                        ps_tr = s2_ps_tr.tile([128, 128], F16, tag="ps_tr")
                        nc.tensor.transpose(ps_tr[:], vT_b[:, j * 128:(j + 1) * 128], ident[:])
                        nc.any.tensor_copy(v_sb[:, j, 0:D], ps_tr[:])

                    for h in range(REP):
                        q_rope = s2_q.tile([128, T], F16, tag="q_rope")
                        load_rope(q_rope, h)

                        # scoresT[s, t] = k_rope.T @ q_rope, exp -> probs (f16)
                        probs = s2_probs.tile([128, NCH, T], F16, tag="probs")
                        for j in range(NCH):
                            t0 = j * 128
                            off = t0
                            for w in _balanced_chunks(T - t0):
                                ps_sc = s2_ps_sc.tile([128, 512], F32, tag="ps_sc")
                                nc.tensor.matmul(
                                    ps_sc[:, 0:w],
                                    k_rope[:, j * 128:(j + 1) * 128],
                                    q_rope[:, off:off + w],
                                    start=True,
                                    stop=True,
                                )
                                nc.scalar.activation(
                                    probs[:, j, off:off + w],
                                    ps_sc[:, 0:w],
                                    mybir.ActivationFunctionType.Exp,
                                )
                                off += w
                            # causal mask on the diagonal block (s > t -> 0)
                            nc.vector.tensor_tensor(
                                probs[:, j, t0:t0 + 128],
                                probs[:, j, t0:t0 + 128],
                                tri_sb[:],
                                mybir.AluOpType.mult,
                            )

                        # PV: out[t, d | sum] = probs.T @ [v | 1]
                        for i in range(NCH):
                            ps_pv = s2_ps_pv.tile([128, D + 1], F32, tag="ps_pv")
                            for j in range(i + 1):
                                nc.tensor.matmul(
                                    ps_pv[:],
                                    probs[:, j, i * 128:(i + 1) * 128],
                                    v_sb[:, j, :],
                                    start=(j == 0),
                                    stop=(j == i),
                                )
                            rcp = s2_tmp.tile([128, 1], F32, tag="rcp")
                            nc.vector.reciprocal(rcp[:], ps_pv[:, D:D + 1])
                            attn_sb = s2_out.tile([128, D], F16, tag="attn")
                            nc.vector.tensor_scalar_mul(attn_sb[:], ps_pv[:, 0:D], rcp[:])
                            ps_tr2 = s2_ps_tr.tile([128, 128], F16, tag="ps_tr")
                            nc.tensor.transpose(ps_tr2[:], attn_sb[:], ident[:])
                            attnT_sb = s2_out.tile([128, 128], F16, tag="attnT")
                            nc.any.tensor_copy(attnT_sb[:], ps_tr2[:])
                            tglob = b * T + i * 128
                            g = tglob // TG
                            goff = tglob % TG
                            nc.sync.dma_start(
                                attnT_dram[g][h * D:(h + 1) * D, goff:goff + 128],
                                attnT_sb[:],
                            )

        # ================= AllGather attnT shards =================
        for g in range(NTG):
            nc.gpsimd.collective_compute(
                "AllGather",
                mybir.AluOpType.bypass,
                replica_groups=[list(range(N_CORES))],
                ins=[attnT_dram[g][:].opt()],
                outs=[attnT_full[g][:].opt()],
            )

        # ================= Stage 3: output projection (transposed) =================
        # outT[co, t] = sum_hd wo[hd, co] * attnT_full[hd, t]
        # wo chunks stationary, attnT moving with FD=1024 (half the matmul
        # instructions of the [t, co] layout); all SBUF loads are per-ko
        # contiguous DMAs spread across the DMA engines.
        KSUB3 = (H * D) // 128  # 32
        with (
            tc.tile_pool(name=f"s3_w{it}", bufs=1) as s3_w,
            tc.tile_pool(name=f"s3_a{it}", bufs=2) as s3_a,
            tc.tile_pool(name=f"s3_ev{it}", bufs=3) as s3_ev,
            tc.tile_pool(name=f"s3_psum{it}", bufs=8, space="PSUM") as s3_psum,
        ):
            wo_sb = s3_w.tile([128, KSUB3, CO_SHARD], F16)
            for k in range(KSUB3):
                nc.sync.dma_start(wo_sb[:, k, :], wo[k * 128:(k + 1) * 128, :])
            for tg in range(NTG):  # 2
                # 8 psum tiles: (4 co chunks) x (2 t halves of 512), each
                # accumulated contiguously over k (moving FD 512 for f16)
                psums3 = [
                    [
                        s3_psum.tile([128, 512], F32,
                                     name=f"s3ps_{it}_{tg}_{m}_{half}", tag="s3ps")
                        for half in range(2)
                    ]
                    for m in range(CO_SHARD // 128)
                ]
                a_sb = s3_a.tile([128, KSUB3, TG], F16, tag="s3a")
                for k in range(KSUB3):
                    nc.sync.dma_start(
                        a_sb[:, k, :],
                        attnT_full[tg][k * 128:(k + 1) * 128, :],
                    )
                for m in range(CO_SHARD // 128):  # 4 co chunks
                    for half in range(2):
                        for k in range(KSUB3):
                            nc.tensor.matmul(
                                psums3[m][half][:],
                                wo_sb[:, k, m * 128:(m + 1) * 128],
                                a_sb[:, k, half * 512:(half + 1) * 512],
                                start=(k == 0),
                                stop=(k == KSUB3 - 1),
                            )
                for m in range(CO_SHARD // 128):
                    ev = s3_ev.tile([128, TG], F16, tag="s3ev")
                    nc.any.tensor_copy(ev[:, 0:512], psums3[m][0][:])
                    nc.any.tensor_copy(ev[:, 512:TG], psums3[m][1][:])
                    nc.sync.dma_start(
                        outT[m * 128:(m + 1) * 128, tg * TG:(tg + 1) * TG],
                        ev[:],
                    )

    nc.compile()
    return nc


def _rope_perm():
    """Column permutation within one head: [0,2,...,126, 1,3,...,127]."""
    return np.concatenate([np.arange(0, D, 2), np.arange(1, D, 2)])


def _host_prep(x, Wq, Wk, Wv, Wo, bo):
    x = np.asarray(x, dtype=np.float32)
    Wq = np.asarray(Wq, dtype=np.float32)
    Wk = np.asarray(Wk, dtype=np.float32)
    Wv = np.asarray(Wv, dtype=np.float32)
    Wo = np.asarray(Wo, dtype=np.float32)

    # cast to fp16 first (halves the bytes moved by the transpose copy)
    xT = np.ascontiguousarray(x.reshape(BT, C).astype(np.float16).T)

    scale = np.float32(D ** -0.5)

    # rope-permute all heads at once; fold the score scale into Wq.
    # perm == [evens | odds], done as reshape+transpose (faster than a gather)
    Wqp = np.ascontiguousarray(
        (Wq * scale).astype(np.float16)
        .reshape(C, H, D // 2, 2).transpose(0, 1, 3, 2).reshape(C, H, D))
    Wkp = np.ascontiguousarray(
        Wk.astype(np.float16)
        .reshape(C, G, D // 2, 2).transpose(0, 1, 3, 2).reshape(C, G, D))
    Wv16 = Wv.astype(np.float16).reshape(C, G, D)
    Wo16 = Wo.astype(np.float16)

    freqs = 1.0 / (ROPE_THETA ** (np.arange(0, D, 2, dtype=np.float64) / D))
    angle = np.arange(T, dtype=np.float64)[:, None] * freqs[None, :]  # [T, 64]
    cosh = np.cos(angle).T.astype(np.float16)   # [64, T]
    sinh = np.sin(angle).T.astype(np.float16)
    cos2 = np.ascontiguousarray(np.vstack([cosh, cosh]))       # [128, T]
    sinpm = np.ascontiguousarray(np.vstack([-sinh, sinh]))     # [128, T]

    sidx = np.arange(128)[:, None]
    tidx = np.arange(128)[None, :]
    tri = np.ascontiguousarray((sidx <= tidx).astype(np.float16))
    ident = np.eye(128, dtype=np.float16)

    in_maps = []
    for c in range(N_CORES):
        in_maps.append({
            "xT": xT,  # full xT replicated to every core (no on-device AllGather)
            "wq": np.ascontiguousarray(
                Wqp[:, c * REP:(c + 1) * REP].reshape(C, HD_SHARD)),
            "wk": np.ascontiguousarray(Wkp[:, c]),
            "wv": np.ascontiguousarray(Wv16[:, c]),
            "wo": np.ascontiguousarray(Wo16[:, c * CO_SHARD:(c + 1) * CO_SHARD]),
            "cos2": cos2,
            "sinpm": sinpm,
            "tri": tri,
            "idin": ident,
        })
    return in_maps


def _get_nc(loop_n=1):
    if loop_n not in _CACHE:
        _CACHE[loop_n] = _build_nc(loop_n)
    return _CACHE[loop_n]


def _run(x, Wq, Wk, Wv, Wo, bo, trace=False, trace_cores=None):
    in_maps = _host_prep(x, Wq, Wk, Wv, Wo, bo)
    nc = _get_nc(1)
    r = bass_utils.run_bass_kernel_spmd(
        nc, in_maps, core_ids=list(range(N_CORES)),
        trace=trace, trace_cores=trace_cores,
    )
    # single-pass assembly: transpose each fp16 [co, t] shard into a
    # preallocated f32 buffer (implicit upcast), add bias in place
    out = np.empty((BT, C), dtype=np.float32)
    for c in range(N_CORES):
        out[:, c * CO_SHARD:(c + 1) * CO_SHARD] = r.results[c]["outT"].T
    out += np.asarray(bo, dtype=np.float32)[None, :]
    return out.reshape(B, T, C), r


def kernel(x, Wq, Wk, Wv, Wo, bo):
    out, _ = _run(x, Wq, Wk, Wv, Wo, bo, trace=False)
    return out


# revision 50
# speedup vs baseline: 1.7421x; 1.7421x over previous
"""GQA attention kernel for 8 Trainium2 NeuronCores (tensor-parallel over heads).

Self-contained: hardcodes shapes from the problem spec.
  x  [2, 1024, 4096]  Wq [4096, 4096]  Wk/Wv [4096, 1024]  Wo [4096, 4096]  bo [4096]
  32 q heads, 8 KV groups, head dim 128, RoPE theta 10000, causal softmax.

Sharding: core c owns KV group c and query heads 4c..4c+3.
  - x is replicated to every core host-side (no on-device AllGather)
  - Wq/Wk/Wv column-sharded per head group (RoPE-permuted, scale folded into Wq)
  - attention computed locally per core; stage 1 and stage 2 interleave per
    batch so the b=0 attnT AllGather overlaps b=1 compute
  - attnT shards AllGathered on-device (one [512,1024] gather per batch),
    Wo column-sharded -> each core returns its 512 output columns
    (transposed); host concatenates, transposes, adds bo.
"""

import numpy as np

import concourse.mybir as mybir
import concourse.tile as tile
from concourse import bacc
from concourse import bass_utils

N_CORES = 8
B, T, C = 2, 1024, 4096
H, G, D = 32, 8, 128
REP = H // G            # q heads per KV group/core = 4
BT = B * T              # 2048
HD_SHARD = REP * D      # 512 q columns per core
CO_SHARD = C // N_CORES  # 512 output columns per core
ROPE_THETA = 10000.0

F32 = mybir.dt.float32
F16 = mybir.dt.float16

_CACHE = {}


def _balanced_chunks(total, maxc=512, quantum=128):
    """Split `total` (multiple of quantum) into chunks <= maxc, balanced."""
    nblk = total // quantum
    n = -(-total // maxc)
    sizes = []
    for i in range(n):
        take = nblk // n + (1 if i < nblk % n else 0)
        sizes.append(take * quantum)
    return sizes


def _build_nc(loop_n=1):
    """Build the kernel. loop_n > 1 wraps the whole body in a hardware
    For_i loop (full all-engine barrier on the back edge) so one dispatch
    runs the kernel loop_n times back-to-back -- used for on-device
    timing; the computed output is identical (idempotent body)."""
    nc = bacc.Bacc("TRN2", target_bir_lowering=False, debug=False, num_devices=N_CORES)

    # ---- I/O ----
    xT = nc.dram_tensor("xT", [C, BT], F16, kind="ExternalInput")
    wq = nc.dram_tensor("wq", [C, HD_SHARD], F16, kind="ExternalInput")
    wk = nc.dram_tensor("wk", [C, D], F16, kind="ExternalInput")
    wv = nc.dram_tensor("wv", [C, D], F16, kind="ExternalInput")
    wo = nc.dram_tensor("wo", [H * D, CO_SHARD], F16, kind="ExternalInput")
    cos2 = nc.dram_tensor("cos2", [D, T], F16, kind="ExternalInput")
    sinpm = nc.dram_tensor("sinpm", [D, T], F16, kind="ExternalInput")
    tri = nc.dram_tensor("tri", [128, 128], F16, kind="ExternalInput")
    idin = nc.dram_tensor("idin", [128, 128], F16, kind="ExternalInput")
    # output stored transposed [co, t]; host transposes back (cheap)
    outT = nc.dram_tensor("outT", [CO_SHARD, BT], F16, kind="ExternalOutput")

    # ---- DRAM scratch ----
    TG = 1024
    NTG = BT // TG  # 2 (one AllGather per batch)
    attnT_dram = [
        nc.dram_tensor(f"attnT_dram_{g}", [HD_SHARD, TG], F16, kind="Internal")
        for g in range(NTG)
    ]
    attnT_full = [
        nc.dram_tensor(f"attnT_full_{g}", [H * D, TG], F16, kind="Internal")
        for g in range(NTG)
    ]

    with tile.TileContext(nc) as tc:
      for it in range(loop_n):  # python-unrolled repeat for on-device timing

        # ================= Stage 1: QKV projection =================
        # qkvT[m, t] = sum_c W[c, m] * xT[c, t];  m = q(512) | k(128) | v(128)
        M_ALL = HD_SHARD + 2 * D  # 768
        M_CHUNKS = M_ALL // 128   # 6 (0-3 q heads, 4 k, 5 v)
        KSUB = C // 128           # 32
        NT = 512                  # t-cols per chunk
        NCH = T // 128  # 8 chunks of both s and t
        with tc.tile_pool(name=f"s1_w{it}", bufs=1) as s1_w:
            # QKV weights stay resident across both batch halves.
            # Per-ko contiguous DMAs spread the load across DMA engines.
            w_sb = s1_w.tile([128, KSUB, M_ALL], F16)
            for k in range(KSUB):
                nc.sync.dma_start(w_sb[:, k, 0:HD_SHARD],
                                  wq[k * 128:(k + 1) * 128, :])
                nc.sync.dma_start(w_sb[:, k, HD_SHARD:HD_SHARD + D],
                                  wk[k * 128:(k + 1) * 128, :])
                nc.sync.dma_start(w_sb[:, k, HD_SHARD + D:M_ALL],
                                  wv[k * 128:(k + 1) * 128, :])
            # qkv stay SBUF-resident between stage 1 and stage 2 (per batch):
            # q heads + k in qk_sb rows [m], v in vT_b. No DRAM round trip.
            qk_sbs = [s1_w.tile([128, REP + 1, T], F16, name=f"qk_{it}_{b}")
                      for b in range(B)]
            vT_bs = [s1_w.tile([128, T], F16, name=f"vT_{it}_{b}")
                     for b in range(B)]

            # Interleave stage 1 and stage 2 per batch: s1(b0) s2(b0) s1(b1)
            # s2(b1).  The b=0 attnT AllGathers then overlap s1(b1)+s2(b1)
            # compute, and stage-2 DVE/ACT work spreads under stage-1 PE.
            for b in range(B):
                # ---- Stage 1 half: QKV projection for this batch ----
                qk_sb = qk_sbs[b]
                vT_b = vT_bs[b]
                with (
                    tc.tile_pool(name=f"s1_x{it}_{b}", bufs=4) as s1_x,
                    tc.tile_pool(name=f"s1_ps{it}_{b}", bufs=8, space="PSUM") as s1_psum,
                ):
                    for n in (2 * b, 2 * b + 1):  # 512-col t chunks
                        psums = [
                            s1_psum.tile([128, NT], F32, name=f"s1ps_{it}_{b}_{m}", tag="s1ps")
                            for m in range(M_CHUNKS)
                        ]
                        for khalf in range(2):
                            x_sb = s1_x.tile([128, KSUB // 2, NT], F16, tag="s1x")
                            for k in range(KSUB // 2):
                                kk = khalf * 16 + k
                                nc.sync.dma_start(
                                    x_sb[:, k, :],
                                    xT[kk * 128:(kk + 1) * 128, n * NT:(n + 1) * NT],
                                )
                            for m in range(M_CHUNKS):
                                for k in range(KSUB // 2):
                                    nc.tensor.matmul(
                                        psums[m][:],
                                        w_sb[:, khalf * 16 + k, m * 128:(m + 1) * 128],
                                        x_sb[:, k, :],
                                        start=(khalf == 0 and k == 0),
                                        stop=(khalf == 1 and k == KSUB // 2 - 1),
                                    )
                        nh = (n % 2) * NT  # col offset within this batch's window
                        # evict on ACT: the DVE queue carries the previous
                        # batch's stage-2 work at this point, and PE stalls
                        # on PSUM slots until evictions run
                        for m in range(M_CHUNKS):
                            if m < M_CHUNKS - 1:
                                nc.scalar.activation(
                                    qk_sb[:, m, nh:nh + NT], psums[m][:],
                                    mybir.ActivationFunctionType.Copy)
                            else:
                                nc.scalar.activation(
                                    vT_b[:, nh:nh + NT], psums[m][:],
                                    mybir.ActivationFunctionType.Copy)

                # ---- Stage 2: attention for this batch ----
                with (
                    tc.tile_pool(name=f"s2_const{it}_{b}", bufs=1) as s2_const,
                    tc.tile_pool(name=f"s2_kv{it}_{b}", bufs=2) as s2_kv,
                    tc.tile_pool(name=f"s2_q{it}_{b}", bufs=3) as s2_q,
                    tc.tile_pool(name=f"s2_tmp{it}_{b}", bufs=3) as s2_tmp,
                    tc.tile_pool(name=f"s2_probs{it}_{b}", bufs=3) as s2_probs,
                    tc.tile_pool(name=f"s2_out{it}_{b}", bufs=3) as s2_out,
                    tc.tile_pool(name=f"s2_ps_sc{it}_{b}", bufs=3, space="PSUM") as s2_ps_sc,
                    tc.tile_pool(name=f"s2_ps_pv{it}_{b}", bufs=2, space="PSUM") as s2_ps_pv,
                    tc.tile_pool(name=f"s2_ps_tr{it}_{b}", bufs=2, space="PSUM") as s2_ps_tr,
                ):
                    ident = s2_const.tile([128, 128], F16)
                    nc.sync.dma_start(ident[:], idin[:])
                    tri_sb = s2_const.tile([128, 128], F16)
                    nc.sync.dma_start(tri_sb[:], tri[:])
                    cos_sb = s2_const.tile([128, T], F16)
                    nc.sync.dma_start(cos_sb[:], cos2[:])
                    sin_sb = s2_const.tile([128, T], F16)
                    nc.sync.dma_start(sin_sb[:], sinpm[:])

                    def load_rope(dst, head):
                        """Rope head rows of SBUF-resident qk_sb into dst.

                        dst = src * [cos;cos] + swap_halves(src) * [-sin;+sin]
                        swap via SBUF->SBUF DMA (crosses partitions).
                        """
                        swp = s2_tmp.tile([128, T], F16, tag="rope_swp")
                        nc.sync.dma_start(swp[0:64], qk_sb[64:128, head, :])
                        nc.sync.dma_start(swp[64:128], qk_sb[0:64, head, :])
                        tmp = s2_tmp.tile([128, T], F16, tag="rope_tmp")
                        nc.vector.tensor_tensor(tmp[:], swp[:], sin_sb[:], mybir.AluOpType.mult)
                        nc.vector.tensor_tensor(dst[:], qk_sb[:, head, :], cos_sb[:], mybir.AluOpType.mult)
                        nc.vector.tensor_tensor(dst[:], dst[:], tmp[:], mybir.AluOpType.add)

                    # k rope
                    k_rope = s2_kv.tile([128, T], F16, tag="k_rope")
                    load_rope(k_rope, REP)
                    # v: transpose SBUF-resident vT to [s, d], append ones col
                    v_sb = s2_kv.tile([128, NCH, D + 1], F16, tag="v_ext")
                    nc.vector.memset(v_sb[:, :, D:D + 1], 1.0)
                    for j in range(NCH):
                        ps_tr = s2_ps_tr.tile([128, 128], F16, tag="ps_tr")
                        nc.tensor.transpose(ps_tr[:], vT_b[:, j * 128:(j + 1) * 128], ident[:])
                        nc.scalar.activation(v_sb[:, j, 0:D], ps_tr[:],
                                             mybir.ActivationFunctionType.Copy)

                    for h in range(REP):
                        q_rope = s2_q.tile([128, T], F16, tag="q_rope")
                        load_rope(q_rope, h)

                        # scoresT[s, t] = k_rope.T @ q_rope, exp -> probs (f16)
                        probs = s2_probs.tile([128, NCH, T], F16, tag="probs")
                        for j in range(NCH):
                            t0 = j * 128
                            off = t0
                            for w in _balanced_chunks(T - t0):
                                ps_sc = s2_ps_sc.tile([128, 512], F32, tag="ps_sc")
                                nc.tensor.matmul(
                                    ps_sc[:, 0:w],
                                    k_rope[:, j * 128:(j + 1) * 128],
                                    q_rope[:, off:off + w],
                                    start=True,
                                    stop=True,
                                )
                                nc.scalar.activation(
                                    probs[:, j, off:off + w],
                                    ps_sc[:, 0:w],
                                    mybir.ActivationFunctionType.Exp,
                                )
                                off += w
                            # causal mask on the diagonal block (s > t -> 0)
                            nc.vector.tensor_tensor(
                                probs[:, j, t0:t0 + 128],
                                probs[:, j, t0:t0 + 128],
                                tri_sb[:],
                                mybir.AluOpType.mult,
                            )

                        # PV: out[t, d | sum] = probs.T @ [v | 1]
                        for i in range(NCH):
                            ps_pv = s2_ps_pv.tile([128, D + 1], F32, tag="ps_pv")
                            for j in range(i + 1):
                                nc.tensor.matmul(
                                    ps_pv[:],
                                    probs[:, j, i * 128:(i + 1) * 128],
                                    v_sb[:, j, :],
                                    start=(j == 0),
                                    stop=(j == i),
                                )
                            rcp = s2_tmp.tile([128, 1], F32, tag="rcp")
                            nc.vector.reciprocal(rcp[:], ps_pv[:, D:D + 1])
                            attn_sb = s2_out.tile([128, D], F16, tag="attn")
                            nc.vector.tensor_scalar_mul(attn_sb[:], ps_pv[:, 0:D], rcp[:])
                            ps_tr2 = s2_ps_tr.tile([128, 128], F16, tag="ps_tr")
                            nc.tensor.transpose(ps_tr2[:], attn_sb[:], ident[:])
                            attnT_sb = s2_out.tile([128, 128], F16, tag="attnT")
                            nc.scalar.activation(attnT_sb[:], ps_tr2[:],
                                                 mybir.ActivationFunctionType.Copy)
                            tglob = b * T + i * 128
                            g = tglob // TG
                            goff = tglob % TG
                            nc.sync.dma_start(
                                attnT_dram[g][h * D:(h + 1) * D, goff:goff + 128],
                                attnT_sb[:],
                            )

        # ================= AllGather attnT shards =================
        for g in range(NTG):
            nc.gpsimd.collective_compute(
                "AllGather",
                mybir.AluOpType.bypass,
                replica_groups=[list(range(N_CORES))],
                ins=[attnT_dram[g][:].opt()],
                outs=[attnT_full[g][:].opt()],
            )

        # ================= Stage 3: output projection (transposed) =================
        # outT[co, t] = sum_hd wo[hd, co] * attnT_full[hd, t]
        # wo chunks stationary, attnT moving with FD=1024 (half the matmul
        # instructions of the [t, co] layout); all SBUF loads are per-ko
        # contiguous DMAs spread across the DMA engines.
        KSUB3 = (H * D) // 128  # 32
        with (
            tc.tile_pool(name=f"s3_w{it}", bufs=1) as s3_w,
            tc.tile_pool(name=f"s3_a{it}", bufs=2) as s3_a,
            tc.tile_pool(name=f"s3_ev{it}", bufs=3) as s3_ev,
            tc.tile_pool(name=f"s3_psum{it}", bufs=8, space="PSUM") as s3_psum,
        ):
            wo_sb = s3_w.tile([128, KSUB3, CO_SHARD], F16)
            for k in range(KSUB3):
                nc.sync.dma_start(wo_sb[:, k, :], wo[k * 128:(k + 1) * 128, :])
            for tg in range(NTG):  # 2
                # 8 psum tiles: (4 co chunks) x (2 t halves of 512), each
                # accumulated contiguously over k (moving FD 512 for f16)
                psums3 = [
                    [
                        s3_psum.tile([128, 512], F32,
                                     name=f"s3ps_{it}_{tg}_{m}_{half}", tag="s3ps")
                        for half in range(2)
                    ]
                    for m in range(CO_SHARD // 128)
                ]
                a_sb = s3_a.tile([128, KSUB3, TG], F16, tag="s3a")
                for k in range(KSUB3):
                    nc.sync.dma_start(
                        a_sb[:, k, :],
                        attnT_full[tg][k * 128:(k + 1) * 128, :],
                    )
                for m in range(CO_SHARD // 128):  # 4 co chunks
                    for half in range(2):
                        for k in range(KSUB3):
                            nc.tensor.matmul(
                                psums3[m][half][:],
                                wo_sb[:, k, m * 128:(m + 1) * 128],
                                a_sb[:, k, half * 512:(half + 1) * 512],
                                start=(k == 0),
                                stop=(k == KSUB3 - 1),
                            )
                for m in range(CO_SHARD // 128):
                    ev = s3_ev.tile([128, TG], F16, tag="s3ev")
                    nc.any.tensor_copy(ev[:, 0:512], psums3[m][0][:])
                    nc.any.tensor_copy(ev[:, 512:TG], psums3[m][1][:])
                    nc.sync.dma_start(
                        outT[m * 128:(m + 1) * 128, tg * TG:(tg + 1) * TG],
                        ev[:],
                    )

    nc.compile()
    return nc


def _rope_perm():
    """Column permutation within one head: [0,2,...,126, 1,3,...,127]."""
    return np.concatenate([np.arange(0, D, 2), np.arange(1, D, 2)])


def _host_prep(x, Wq, Wk, Wv, Wo, bo):
    x = np.asarray(x, dtype=np.float32)
    Wq = np.asarray(Wq, dtype=np.float32)
    Wk = np.asarray(Wk, dtype=np.float32)
    Wv = np.asarray(Wv, dtype=np.float32)
    Wo = np.asarray(Wo, dtype=np.float32)

    # cast to fp16 first (halves the bytes moved by the transpose copy)
    xT = np.ascontiguousarray(x.reshape(BT, C).astype(np.float16).T)

    scale = np.float32(D ** -0.5)

    # rope-permute all heads at once; fold the score scale into Wq.
    # perm == [evens | odds], done as reshape+transpose (faster than a gather)
    Wqp = np.ascontiguousarray(
        (Wq * scale).astype(np.float16)
        .reshape(C, H, D // 2, 2).transpose(0, 1, 3, 2).reshape(C, H, D))
    Wkp = np.ascontiguousarray(
        Wk.astype(np.float16)
        .reshape(C, G, D // 2, 2).transpose(0, 1, 3, 2).reshape(C, G, D))
    Wv16 = Wv.astype(np.float16).reshape(C, G, D)
    Wo16 = Wo.astype(np.float16)

    freqs = 1.0 / (ROPE_THETA ** (np.arange(0, D, 2, dtype=np.float64) / D))
    angle = np.arange(T, dtype=np.float64)[:, None] * freqs[None, :]  # [T, 64]
    cosh = np.cos(angle).T.astype(np.float16)   # [64, T]
    sinh = np.sin(angle).T.astype(np.float16)
    cos2 = np.ascontiguousarray(np.vstack([cosh, cosh]))       # [128, T]
    sinpm = np.ascontiguousarray(np.vstack([-sinh, sinh]))     # [128, T]

    sidx = np.arange(128)[:, None]
    tidx = np.arange(128)[None, :]
    tri = np.ascontiguousarray((sidx <= tidx).astype(np.float16))
    ident = np.eye(128, dtype=np.float16)

    in_maps = []
    for c in range(N_CORES):
        in_maps.append({
            "xT": xT,  # full xT replicated to every core (no on-device AllGather)
            "wq": np.ascontiguousarray(
                Wqp[:, c * REP:(c + 1) * REP].reshape(C, HD_SHARD)),
            "wk": np.ascontiguousarray(Wkp[:, c]),
            "wv": np.ascontiguousarray(Wv16[:, c]),
            "wo": np.ascontiguousarray(Wo16[:, c * CO_SHARD:(c + 1) * CO_SHARD]),
            "cos2": cos2,
            "sinpm": sinpm,
            "tri": tri,
            "idin": ident,
        })
    return in_maps


def _get_nc(loop_n=1):
    if loop_n not in _CACHE:
        _CACHE[loop_n] = _build_nc(loop_n)
    return _CACHE[loop_n]


def _run(x, Wq, Wk, Wv, Wo, bo, trace=False, trace_cores=None):
    in_maps = _host_prep(x, Wq, Wk, Wv, Wo, bo)
    nc = _get_nc(1)
    r = bass_utils.run_bass_kernel_spmd(
        nc, in_maps, core_ids=list(range(N_CORES)),
        trace=trace, trace_cores=trace_cores,
    )
    # single-pass assembly: transpose each fp16 [co, t] shard into a
    # preallocated f32 buffer (implicit upcast), add bias in place
    out = np.empty((BT, C), dtype=np.float32)
    for c in range(N_CORES):
        out[:, c * CO_SHARD:(c + 1) * CO_SHARD] = r.results[c]["outT"].T
    out += np.asarray(bo, dtype=np.float32)[None, :]
    return out.reshape(B, T, C), r


def kernel(x, Wq, Wk, Wv, Wo, bo):
    out, _ = _run(x, Wq, Wk, Wv, Wo, bo, trace=False)
    return out
